# revision 1
# baseline (speedup 1.0000x reference)
"""Trainium2 Bass kernel for DCRNN-Temporal (gnn_message_passing).

Contract: kernel(**inputs) takes FULL numpy inputs (x, edge_index,
edge_weight, w_z, b_z, w_r, b_r, w_h, b_h, w_lin, b_lin) and returns the
FULL [N, 12] output, running a Bass SPMD kernel on 8 NeuronCores.

Math (H0 = 0 simplifies the DCRNN cell):
  R is unused (H0*R = 0), both remaining DConvs share the same diffusion
  features. With per-edge weights 1/deg(src) folded into pre-scaled
  tables and the Chebyshev recurrence folded into effective weights:
    T1o = P_f x, T1i = P_r x, Y2o = P_f T1o, Y2i = P_r T1i
    G   = [x | T1o, Y2o, T1i, Y2i] @ Weff + beff    (z | h gates)
    H   = sigmoid(-Gz - bz) * tanh(Gh + bh)         (= (1-Z)*Htilde)
    out = relu(H) @ w_lin + b_lin

Sharding: nodes partitioned by destination across 8 cores.  Each
propagate = dma_gather row gathers (256B rows) + DVE adds over
degree-sorted ELL rounds (scatter-free).  dma_gather indices are int16,
so every node table uses one global 6256-periodic layout (6250 node
rows + 6 zero rows per core; gid(v) = (v//6250)*6256 + v%6250 < 50048)
and each direction's edges split into two windowed streams:
  A: gid < 32768 (idx = gid),  B: idx = gid - 32768 (table AP offset).
Streams accumulate in their own degree-sorted order; dma_scatter_add
(unique int16 local-node targets) merges them back to natural order in
DRAM.  One AllGather exchanges the scaled T1 tables between hops.
"""

import os
import sys

for _p in ("/opt/trn_rl_repo", "/root/.axon_site/_ro/trn_rl_repo"):
    if os.path.isdir(_p) and _p not in sys.path:
        sys.path.insert(0, _p)
        break

import numpy as np

import concourse.bass as bass
import concourse.tile as tile
from concourse import bacc, mybir
from concourse import bass_utils
from concourse.masks import make_identity

F = 32          # node feature dim
FO = 64         # out channels per gate
GATES = 2 * FO
PER = 12        # head output dim
NCORES = 8
ES = 64         # table row length (f32) = 256B, required by dma_gather
CH = 7          # SWDGE chunk width in 128-cols (<=896 descriptors/call)
NT = 512        # node tile (free dim) for gate/head matmuls
WINDOW = 32768  # int16 idx window
GAPZ = 6        # zero rows appended per core in the global table layout

FP32 = mybir.dt.float32
I16 = mybir.dt.int16


def _ceil(a, b):
    return -(-a // b)


class _StageDone(Exception):
    pass


def _wrap16(idx_seq):
    """[n] -> [128, n//16] int16, wrapped in 16 partitions, replicated 8x."""
    w = idx_seq.reshape(-1, 16).T.astype(np.int16)
    return np.ascontiguousarray(np.tile(w, (8, 1)))


# ----------------------------------------------------------------------------
# Host-side graph preprocessing
# ----------------------------------------------------------------------------

def _build_streams(dst, src_gid, N, NLOC, P, W):
    """ELL-round structures for one directed edge set, split into
    idx-window streams A (gid < W) and B (gid >= W)."""
    core = dst // NLOC
    dl = dst - core * NLOC
    streams = []
    for s, mask in (("A", src_gid < W), ("B", src_gid >= W)):
        percore = []
        for p in range(P):
            sel = np.nonzero((core == p) & mask)[0]
            cnt = np.bincount(dl[sel], minlength=NLOC)
            perm = np.argsort(-cnt, kind="stable").astype(np.int32)
            rank = np.empty(NLOC, np.int32)
            rank[perm] = np.arange(NLOC, dtype=np.int32)
            order = np.argsort(dl[sel], kind="stable")
            es = sel[order]
            ptr = np.zeros(NLOC + 1, np.int64)
            np.cumsum(cnt, out=ptr[1:])
            j_arr = np.arange(len(es), dtype=np.int64) - ptr[dl[es]]
            percore.append(dict(es=es, j=j_arr, r=rank[dl[es]].astype(np.int64),
                                cnt=cnt, perm=perm))
        maxdeg = int(max(pc["cnt"].max(initial=0) for pc in percore))
        NJ = []
        for j in range(maxdeg):
            nj = max(int((pc["cnt"] > j).sum()) for pc in percore)
            NJ.append(_ceil(nj, 128) * 128)
        off = np.zeros(len(NJ) + 1, np.int64)
        np.cumsum(NJ, out=off[1:])
        EP = int(off[-1])
        for pc in percore:
            pc["slot"] = off[pc["j"]] + pc["r"]
        streams.append(dict(name=s, percore=percore, NJ=NJ, off=off, EP=EP))
    return streams


def preprocess(x, edge_index, edge_weight, w_z, b_z, w_r, b_r, w_h, b_h,
               w_lin, b_lin, P=NCORES, window=WINDOW):
    N, Fx = x.shape
    assert Fx == F
    assert N % P == 0
    NLOC = N // P
    NLOCP = _ceil(NLOC, 128) * 128
    CACC = NLOCP // 128
    NL6 = NLOC + GAPZ
    NTAB = P * NL6
    W = min(window, NTAB)
    assert NTAB - W < 32768 and NLOC < W

    row = np.asarray(edge_index[0], dtype=np.int64)
    col = np.asarray(edge_index[1], dtype=np.int64)
    ew = np.asarray(edge_weight, dtype=np.float64)
    deg_out = np.bincount(row, weights=ew, minlength=N)
    deg_in = np.bincount(col, weights=ew, minlength=N)
    with np.errstate(divide="ignore"):
        doi = np.where(deg_out > 0, 1.0 / deg_out, 0.0).astype(np.float32)
        dii = np.where(deg_in > 0, 1.0 / deg_in, 0.0).astype(np.float32)

    gid = (np.arange(N) // NLOC) * NL6 + (np.arange(N) % NLOC)
    xf = np.asarray(x, np.float32)

    def table(scaled):
        t = np.zeros((NTAB, ES), np.float32)
        t[gid, :F] = scaled
        return t

    XF = table(xf * doi[:, None])
    XR = table(xf * dii[:, None])

    # pad slot targets: a zero row inside each window
    apad = NLOC  # core-0 zero row, < W
    zq = _ceil(W - NLOC, NL6)  # first core whose zero row is >= W
    bpad = zq * NL6 + NLOC - W
    assert 0 <= bpad < NTAB - W or W == NTAB

    gsrcF = gid[row]
    gsrcR = gid[col]
    stF = _build_streams(col, gsrcF, N, NLOC, P, W)
    stR = _build_streams(row, gsrcR, N, NLOC, P, W)

    # effective gate weights (K = 3); WA rows = [T1o, Y2o, T1i, Y2i]
    assert w_z.shape[1] == 3

    def gate_w(w):
        w = np.asarray(w, np.float32)
        wx = w[0, 0, :F] + w[1, 0, :F] - w[0, 2, :F] - w[1, 2, :F]
        wa = np.concatenate(
            [w[0, 1, :F], 2.0 * w[0, 2, :F], w[1, 1, :F], 2.0 * w[1, 2, :F]], 0)
        return wx, wa

    wxz, waz = gate_w(w_z)
    wxh, wah = gate_w(w_h)
    WA = np.ascontiguousarray(np.concatenate([waz, wah], axis=1))
    WX = np.ascontiguousarray(np.concatenate([wxz, wxh], axis=1))
    biasS = np.ascontiguousarray(-np.asarray(b_z, np.float32)[:, None])
    biasT = np.ascontiguousarray(np.asarray(b_h, np.float32)[:, None])
    WL = np.asarray(w_lin, np.float32)
    BL = np.ascontiguousarray(np.asarray(b_lin, np.float32)[:, None])

    def expand(vec):  # [NLOCP] -> [128, CACC, F] accumulator-layout expand
        return np.ascontiguousarray(
            np.repeat(vec.reshape(CACC, 128).T, F, axis=1)
        ).reshape(128, CACC, F)

    in_maps = []
    for p in range(P):
        m = {"xf": XF, "xr": XR, "wa": WA, "wx": WX, "biass": biasS,
             "biast": biasT, "wl": WL, "bl": BL}
        for dname, st, gsrc, srcv in (("f", stF, gsrcF, row),
                                      ("r", stR, gsrcR, col)):
            for s in st:
                pc = s["percore"][p]
                if s["EP"] == 0:
                    continue
                base, pad = (0, apad) if s["name"] == "A" else (W, bpad)
                ivals = np.full(s["EP"], pad, np.int64)
                ivals[pc["slot"]] = gsrc[pc["es"]] - base
                m[f"ix{dname}{s['name'].lower()}"] = _wrap16(ivals)
                sidx = np.full(NLOCP, NLOC, np.int64)
                sidx[:NLOC] = pc["perm"]
                m[f"sx{dname}{s['name'].lower()}"] = _wrap16(sidx)
        sl = slice(p * NLOC, (p + 1) * NLOC)
        xp = np.zeros((NLOCP, F), np.float32)
        xp[:NLOC] = xf[sl]
        m["xpt"] = np.ascontiguousarray(xp.T)
        tmp = np.zeros(NLOCP, np.float32)
        tmp[:NLOC] = doi[sl]
        m["doe"] = expand(tmp)
        tmp = np.zeros(NLOCP, np.float32)
        tmp[:NLOC] = dii[sl]
        m["die"] = expand(tmp)
        in_maps.append(m)

    meta = dict(N=N, P=P, NLOC=NLOC, NLOCP=NLOCP, CACC=CACC, NL6=NL6,
                NTAB=NTAB, W=W,
                streams={"f": [dict(name=s["name"], NJ=s["NJ"], off=s["off"],
                                    EP=s["EP"]) for s in stF],
                         "r": [dict(name=s["name"], NJ=s["NJ"], off=s["off"],
                                    EP=s["EP"]) for s in stR]})
    return in_maps, meta


# ----------------------------------------------------------------------------
# Device program
# ----------------------------------------------------------------------------

def build_program(meta, debug=False):
    from contextlib import ExitStack

    stage = int(os.environ.get("KSTAGE", "9"))
    konly = os.environ.get("KONLY", "")
    nogath = os.environ.get("KNOGATH", "")
    noadd = os.environ.get("KNOADD", "")
    noscat = os.environ.get("KNOSCAT", "")

    N, P = meta["N"], meta["P"]
    NLOC, NLOCP, CACC = meta["NLOC"], meta["NLOCP"], meta["CACC"]
    NL6, NTAB, W = meta["NL6"], meta["NTAB"], meta["W"]
    streams = meta["streams"]

    nc = bacc.Bacc("TRN2", target_bir_lowering=False, debug=False,
                   num_devices=P)

    def din(name, shape, dt=FP32):
        return nc.dram_tensor(name, list(shape), dt, kind="ExternalInput").ap()

    def dout(name, shape, dt=FP32):
        return nc.dram_tensor(name, list(shape), dt, kind="ExternalOutput").ap()

    xf_d = din("xf", (NTAB, ES))
    xr_d = din("xr", (NTAB, ES))
    ix_d, sx_d = {}, {}
    for d in ("f", "r"):
        for s in streams[d]:
            if s["EP"] == 0:
                continue
            k = d + s["name"].lower()
            ix_d[k] = din("ix" + k, (128, s["EP"] // 16), I16)
            sx_d[k] = din("sx" + k, (128, NLOCP // 16), I16)
    xpt_d = din("xpt", (F, NLOCP))
    doe_d = din("doe", (128, CACC, F))
    die_d = din("die", (128, CACC, F))
    wa_d = din("wa", (4 * F, GATES))
    wx_d = din("wx", (F, GATES))
    biass_d = din("biass", (FO, 1))
    biast_d = din("biast", (FO, 1))
    wl_d = din("wl", (FO, PER))
    bl_d = din("bl", (PER, 1))
    out_d = dout("out", (PER, NLOCP))
    dbg = {}
    if debug:
        dbg["dyall"] = dout("dyall", (128, CACC * 4 * F))
        dbg["dbounf"] = dout("dbounf", (NLOCP, ES))
        dbg["dbounr"] = dout("dbounr", (NLOCP, ES))
        dbg["dag"] = dout("dag", (NTAB, 2 * ES))

    EPmax = max(s["EP"] for d in ("f", "r") for s in streams[d])

    with tile.TileContext(nc) as tc, ExitStack() as ctx:
      try:
          sb = ctx.enter_context(tc.tile_pool(name="sb", bufs=1))
          ya = ctx.enter_context(tc.tile_pool(name="ya", bufs=3))
          gp = ctx.enter_context(tc.tile_pool(name="gp", bufs=2))
          ixp = ctx.enter_context(tc.tile_pool(name="ixp", bufs=2))
          big = ctx.enter_context(tc.tile_pool(name="big", bufs=2))
          px = ctx.enter_context(tc.tile_pool(name="px", bufs=2))
          pp = ctx.enter_context(tc.tile_pool(name="pp", bufs=2, space="PSUM"))
          pt = ctx.enter_context(tc.tile_pool(name="pt", bufs=2, space="PSUM"))
          dr = ctx.enter_context(tc.tile_pool(name="dr", bufs=1, space="DRAM"))

          phiA = sb.tile([128, NLOCP], FP32, tag="phiA")
          doe = sb.tile([128, CACC, F], FP32, tag="doe")
          die = sb.tile([128, CACC, F], FP32, tag="die")
          sxt = {}
          for d in ("f", "r"):
              for s in streams[d]:
                  if s["EP"] == 0:
                      continue
                  k = d + s["name"].lower()
                  sxt[k] = sb.tile([128, NLOCP // 16], I16, tag="sx" + k,
                                   name="sx" + k)
          wa = sb.tile([4 * F, GATES], FP32, tag="wa")
          wx = sb.tile([F, GATES], FP32, tag="wx")
          bS = sb.tile([FO, 1], FP32, tag="bS")
          bT = sb.tile([FO, 1], FP32, tag="bT")
          wl = sb.tile([FO, PER], FP32, tag="wl")
          bl = sb.tile([PER, 1], FP32, tag="bl")
          ident = sb.tile([128, 128], FP32, tag="ident")

          NBNC = _ceil(max(NLOCP, NL6), 128) * 128
          CB = NBNC // 128
          bounF = dr.tile([NBNC, ES], FP32, tag="bounF")
          bounR = dr.tile([NBNC, ES], FP32, tag="bounR")
          bounS = dr.tile([NBNC, 2 * ES], FP32, tag="bounS")
          scrF = dr.tile([NBNC, ES], FP32, tag="scrF")
          scrR = dr.tile([NBNC, ES], FP32, tag="scrR")
          ag = dr.tile([NTAB, 2 * ES], FP32, tag="ag")
          ag2 = dr.tile([NTAB, 2 * ES], FP32, tag="ag2")

          # --- loads --------------------------------------------------------
          for k, t in sxt.items():
              nc.sync.dma_start(t[:], sx_d[k])
          nc.sync.dma_start(doe[:], doe_d)
          nc.sync.dma_start(die[:], die_d)
          nc.sync.dma_start(wa[:], wa_d)
          nc.sync.dma_start(wx[:], wx_d)
          nc.sync.dma_start(bS[:], biass_d)
          nc.sync.dma_start(bT[:], biast_d)
          nc.sync.dma_start(wl[:], wl_d)
          nc.sync.dma_start(bl[:], bl_d)
          make_identity(nc, ident[:])

          def wrap(dram_tile, c0, cn):
              apv = dram_tile[:].rearrange("(c p) f -> p c f", p=128)
              return apv[:, :, c0:c0 + cn]

          # --- prezero DRAM scratch from a zeroed tile ----------------------
          z0 = ya.tile([128, CB, ES], FP32, tag="y", name="z0")
          nc.vector.memset(z0[:], 0.0)
          nc.sync.dma_start(wrap(bounF, 0, ES), z0[:])
          nc.sync.dma_start(wrap(bounR, 0, ES), z0[:])
          nc.sync.dma_start(wrap(scrF, 0, ES), z0[:])
          nc.sync.dma_start(wrap(scrR, 0, ES), z0[:])
          nc.sync.dma_start(wrap(bounS, 0, ES), z0[:])
          nc.sync.dma_start(wrap(bounS, ES, ES), z0[:])

          # --- one windowed-stream propagate --------------------------------
          qn = [0]

          def prop_stream(yt, sdesc, ixkey, table_ap, estep):
              EP = sdesc["EP"]
              EPc = EP // 128
              roff = [int(o) // 128 for o in sdesc["off"]]
              nrounds = len(sdesc["NJ"])
              ixt = ixp.tile([128, EPmax // 16], I16, tag="ix")
              nc.sync.dma_start(ixt[:, :EP // 16], ix_d[ixkey])
              for c0 in range(0, EPc, CH):
                  c1 = min(c0 + CH, EPc)
                  g = gp.tile([128, CH, ES], FP32, tag="g")
                  if nogath:
                      nc.vector.memset(g[:], 0.0)
                  else:
                      nc.gpsimd.dma_gather(
                          out_ap=g[:, :c1 - c0, :],
                          in_ap=table_ap,
                          idxs_ap=ixt[:, c0 * 8:c1 * 8],
                          num_idxs=(c1 - c0) * 128,
                          num_idxs_reg=(c1 - c0) * 128,
                          elem_size=ES,
                          elem_step=estep,
                      )
                  if noadd:
                      continue
                  for j in range(nrounds):
                      s = max(roff[j], c0)
                      e = min(roff[j + 1], c1)
                      if s >= e:
                          continue
                      ys = s - roff[j]
                      nc.vector.tensor_tensor(
                          out=yt[:, ys:ys + e - s, 0:F],
                          in0=yt[:, ys:ys + e - s, 0:F],
                          in1=g[:, s - c0:e - c0, 0:F],
                          op=mybir.AluOpType.add,
                      )

          def hop(d, tabA, tabB, estep, scr_tile):
              """Run both streams of direction d's propagate; merge-scatter
              into scr_tile (natural order)."""
              for s in streams[d]:
                  if s["EP"] == 0:
                      continue
                  k = d + s["name"].lower()
                  if konly and k not in konly.split(","):
                      continue
                  yt = ya.tile([128, CACC, ES], FP32, tag="y")
                  nc.vector.memset(yt[:], 0.0)
                  prop_stream(yt, s, k, tabA if s["name"] == "A" else tabB,
                              estep)
                  if noscat:
                      continue
                  for c0 in range(0, CACC, CH):
                      c1 = min(c0 + CH, CACC)
                      nc.gpsimd.dma_scatter_add(
                          out_ap=scr_tile[:],
                          in_ap=yt[:, c0:c1, :],
                          idxs_ap=sxt[k][:, c0 * 8:c1 * 8],
                          num_idxs=(c1 - c0) * 128,
                          num_idxs_reg=(c1 - c0) * 128,
                          elem_size=ES,
                      )

          # hop 1 (tables xf/xr, row stride ES)
          hop("f", xf_d[0:W, :], xf_d[W:NTAB, :], ES, bounF)
          hop("r", xr_d[0:W, :], xr_d[W:NTAB, :], ES, bounR)
          if debug:
              nc.sync.dma_start(dbg["dbounf"], bounF[0:NLOCP, :])
              nc.sync.dma_start(dbg["dbounr"], bounR[0:NLOCP, :])
          if stage <= 1:
              raise _StageDone(nc)

          yall = big.tile([128, CACC, 4 * F], FP32, tag="big")
          # T1 natural -> yall blocks 0 (T1o), 2 (T1i); scale -> bounS
          nc.sync.dma_start(yall[:, :, 0:F], wrap(bounF, 0, F)[:, :CACC, :])
          nc.sync.dma_start(yall[:, :, 2 * F:3 * F],
                            wrap(bounR, 0, F)[:, :CACC, :])
          ts = gp.tile([128, CACC, F], FP32, tag="g")
          nc.vector.tensor_tensor(out=ts[:], in0=yall[:, :, 0:F], in1=doe[:],
                                  op=mybir.AluOpType.mult)
          nc.scalar.dma_start(wrap(bounS, 0, F)[:, :CACC, :], ts[:])
          ts2 = gp.tile([128, CACC, F], FP32, tag="g")
          nc.vector.tensor_tensor(out=ts2[:], in0=yall[:, :, 2 * F:3 * F],
                                  in1=die[:], op=mybir.AluOpType.mult)
          nc.scalar.dma_start(wrap(bounS, ES, F)[:, :CACC, :], ts2[:])

          if stage <= 2:
              raise _StageDone(nc)
          nc.gpsimd.collective_compute(
              "AllGather", mybir.AluOpType.bypass,
              replica_groups=[list(range(P))],
              ins=[bounS[0:NL6, :].opt()],
              outs=[ag[0:NTAB, :].opt()],
          )
          if debug:
              nc.sync.dma_start(dbg["dag"], ag[:])
          # Q7-ucode (SWDGE) readers do not sync correctly against a
          # collective's output; interpose a HWDGE copy (proven-good sync
          # on both edges) and gather from the copy.
          nc.sync.dma_start(ag2[:], ag[:])
          if stage <= 3:
              raise _StageDone(nc)

          # hop 2 (table ag2, fwd cols 0:ES rev cols ES:2ES, row stride 2*ES)
          hop("f", ag2[0:W, 0:ES], ag2[W:NTAB, 0:ES], 2 * ES, scrF)
          hop("r", ag2[0:W, ES:2 * ES], ag2[W:NTAB, ES:2 * ES], 2 * ES, scrR)

          # Y2 natural -> yall blocks 1 (Y2o), 3 (Y2i)
          nc.sync.dma_start(yall[:, :, F:2 * F], wrap(scrF, 0, F)[:, :CACC, :])
          nc.sync.dma_start(yall[:, :, 3 * F:4 * F],
                            wrap(scrR, 0, F)[:, :CACC, :])

          if debug:
              nc.scalar.dma_start(dbg["dyall"],
                                  yall[:].rearrange("p c f -> p (c f)"))

          # --- transpose into phiA (feature-major) --------------------------
          for t in range(CACC):
              ps = pt.tile([128, 128], FP32, tag="ps")
              nc.tensor.transpose(out=ps[:], in_=yall[:, t, :],
                                  identity=ident[:])
              nc.vector.tensor_copy(out=phiA[:, t * 128:(t + 1) * 128],
                                    in_=ps[:])

          # --- gate matmuls + activations -----------------------------------
          gZ = big.tile([FO, NLOCP], FP32, tag="big")
          gH = big.tile([FO, NLOCP], FP32, tag="big")
          for n0 in range(0, NLOCP, NT):
              n1 = min(n0 + NT, NLOCP)
              pxt = px.tile([F, NT], FP32, tag="px")
              nc.sync.dma_start(pxt[:, :n1 - n0], xpt_d[:, n0:n1])
              pgz = pp.tile([FO, NT], FP32, tag="pgz")
              nc.tensor.matmul(out=pgz[:, :n1 - n0], lhsT=wa[:, 0:FO],
                               rhs=phiA[:, n0:n1], start=True, stop=False)
              nc.tensor.matmul(out=pgz[:, :n1 - n0], lhsT=wx[:, 0:FO],
                               rhs=pxt[:, :n1 - n0], start=False, stop=True)
              nc.vector.tensor_copy(out=gZ[:, n0:n1], in_=pgz[:, :n1 - n0])
              pgh = pp.tile([FO, NT], FP32, tag="pgh")
              nc.tensor.matmul(out=pgh[:, :n1 - n0], lhsT=wa[:, FO:GATES],
                               rhs=phiA[:, n0:n1], start=True, stop=False)
              nc.tensor.matmul(out=pgh[:, :n1 - n0], lhsT=wx[:, FO:GATES],
                               rhs=pxt[:, :n1 - n0], start=False, stop=True)
              nc.vector.tensor_copy(out=gH[:, n0:n1], in_=pgh[:, :n1 - n0])

          AF = mybir.ActivationFunctionType
          nc.scalar.activation(out=gZ[:], in_=gZ[:],
                               func=AF.Sigmoid, bias=bS[:], scale=-1.0)
          nc.scalar.activation(out=gH[:], in_=gH[:],
                               func=AF.Tanh, bias=bT[:], scale=1.0)
          nc.vector.tensor_tensor(out=gZ[:], in0=gZ[:], in1=gH[:],
                                  op=mybir.AluOpType.mult)
          nc.vector.tensor_scalar_max(gZ[:], gZ[:], 0.0)

          # --- head ---------------------------------------------------------
          for n0 in range(0, NLOCP, NT):
              n1 = min(n0 + NT, NLOCP)
              po = pp.tile([PER, NT], FP32, tag="po")
              nc.tensor.matmul(out=po[:, :n1 - n0], lhsT=wl[:],
                               rhs=gZ[:, n0:n1], start=True, stop=True)
              ot = px.tile([PER, NT], FP32, tag="ot")
              nc.scalar.add(out=ot[:, :n1 - n0], in_=po[:, :n1 - n0], add=bl[:])
              nc.scalar.dma_start(out_d[:, n0:n1], ot[:, :n1 - n0])

      except _StageDone:
          pass
    nc.compile()
    return nc


# ----------------------------------------------------------------------------
# Entry point
# ----------------------------------------------------------------------------

def _assemble(results, meta):
    N, P, NLOC = meta["N"], meta["P"], meta["NLOC"]
    out = np.empty((N, PER), np.float32)
    for p in range(P):
        out[p * NLOC:(p + 1) * NLOC] = results[p]["out"].T[:NLOC]
    return out


def kernel(x, edge_index, edge_weight, w_z, b_z, w_r, b_r, w_h, b_h,
           w_lin, b_lin, _trace=False, _window=WINDOW):
    in_maps, meta = preprocess(x, edge_index, edge_weight, w_z, b_z, w_r,
                               b_r, w_h, b_h, w_lin, b_lin, window=_window)
    nc = build_program(meta)
    res = bass_utils.run_bass_kernel_spmd(
        nc, in_maps, core_ids=list(range(meta["P"])), trace=_trace)
    out = _assemble(res.results, meta)
    if _trace:
        return out, res
    return out

